# revision 3
# baseline (speedup 1.0000x reference)
"""Trainium2 Bass kernel: Mistral quantized MLP (SwiGLU with int8-valued int32
weights, per-output-channel scales).

  gate = (x @ dequant(gate_wq).T), up = (x @ dequant(up_wq).T)
  h = silu(gate) * up
  out = h @ dequant(down_wq).T

Strategy (8 NeuronCores, tensor-parallel on the intermediate dim I):
  - Core c owns rows [c*I/8, (c+1)*I/8) of gate/up and the matching columns of
    down. Each core computes a full [H, T] partial of the down projection;
    the host sums the 8 partials (the "all-reduce"), applies down_scale, and
    transposes back to [B, S, H].

  - Precision: 3-pass fp8 (e4m3) with DoubleRow matmuls (2x PE rate).
    The int8 weights split EXACTLY into two e4m3 halves:
        w = 16*w_hi + w_lo,  w_hi = round(w/16) in [-8,8], w_lo in [-8,8)
    Device stationary variants (both exact in e4m3):
        w_hi2 = 2*w_hi  (= W_HI/8)   and   w_lo8 = w_lo/8
    Activations split into an fp8 hi/lo pair at scale 8:
        a_hi8 = e4m3(8*a),  a_lo8 = e4m3(8*a - a_hi8)
    Then  a @ w  ~=  (a_hi8 + a_lo8) @ w_hi2 + a_hi8 @ w_lo8   (PSUM f32),
    dropping only the tiny a_lo*w_lo cross term. Three fp8 DoubleRow passes
    cost 0.75x of one fp16 pass; end-to-end rel err ~3e-3 (vs 2e-2 budget).

  - Device layout keeps features on partitions, tokens on the free dim:
    x is pre-split/tiled on the host to [mega, ki, 2(hi/lo), ko, t]; weights
    to [o_tile, ki, 2(hi2/lo8), ko, o]. Each DoubleRow matmul consumes a
    pair of k-tiles: lhsT=[128, 2, 128] stationary, rhs=[128, 2, 512]
    moving, out=[128, 512] fp32 PSUM at 256 cycles.

  - h = silu(gate)*up is computed in f32, then split on device:
    h_hi8 = e4m3(8h) (scalar engine, Copy activation w/ scale=8),
    h_lo8 = e4m3(8h - h_hi8) (vector engine, scalar_tensor_tensor).
"""

import os

import ml_dtypes
import numpy as np

_E4 = ml_dtypes.float8_e4m3

# Problem dims (hardcoded per the task contract).
B, S, H, I = 2, 2048, 4096, 14336
NCORES = 8
I_LOC = I // NCORES  # 1792
T = B * S  # 4096
T_MEGA = 512  # tokens per resident x block (and per-matmul free dim)

_nc_cache = {}


def _build_module(t_mega, n_mega, ko_g, ot_g, ot_d, act_name="Silu"):
    """Build + compile the (SPMD, identical on all cores) Bass module.

    ko_g: contraction tiles for gate/up (H/128), must be even
    ot_g: output tiles per core for gate/up (I_loc/128); also the down
          contraction tile count, must be even
    ot_d: output tiles for down (H/128)
    """
    import concourse.tile as tile
    from concourse import bacc, mybir

    f32 = mybir.dt.float32
    f8 = mybir.dt.float8e4
    silu = getattr(mybir.ActivationFunctionType, act_name)
    copy_act = mybir.ActivationFunctionType.Copy
    mult = mybir.AluOpType.mult
    sub = mybir.AluOpType.subtract
    DR = mybir.MatmulPerfMode.DoubleRow
    ko_d = ot_g
    assert ko_g % 2 == 0 and ko_d % 2 == 0

    nc = bacc.Bacc(
        "TRN2",
        target_bir_lowering=False,
        debug=False,
        enable_asserts=False,
        num_devices=NCORES,
    )

    x_d = nc.dram_tensor(
        "x8", [n_mega, 128, 2, ko_g, t_mega], f8, kind="ExternalInput"
    ).ap()
    gw_d = nc.dram_tensor(
        "gate_w", [ot_g, 128, 2, ko_g, 128], f8, kind="ExternalInput"
    ).ap()
    uw_d = nc.dram_tensor(
        "up_w", [ot_g, 128, 2, ko_g, 128], f8, kind="ExternalInput"
    ).ap()
    dw_d = nc.dram_tensor(
        "down_w", [ot_d, 128, 2, ko_d, 128], f8, kind="ExternalInput"
    ).ap()
    gs_d = nc.dram_tensor("gate_s", [128, ot_g], f32, kind="ExternalInput").ap()
    us_d = nc.dram_tensor("up_s", [128, ot_g], f32, kind="ExternalInput").ap()
    out_d = nc.dram_tensor(
        "out", [ot_d * 128, n_mega * t_mega], f32, kind="ExternalOutput"
    ).ap()

    with tile.TileContext(nc) as tc:
        with (
            tc.tile_pool(name="px", bufs=2) as px,
            tc.tile_pool(name="pw", bufs=2) as pw,
            tc.tile_pool(name="pdw", bufs=4) as pdw,
            tc.tile_pool(name="ph", bufs=2) as ph,
            tc.tile_pool(name="pe", bufs=2) as pe,
            tc.tile_pool(name="po", bufs=3) as po,
            tc.tile_pool(name="pscale", bufs=1) as pscale,
            tc.tile_pool(name="pp", bufs=8, space="PSUM") as pp,
        ):
            gs_t = pscale.tile([128, ot_g], f32, name="gs_t")
            nc.sync.dma_start(out=gs_t[:], in_=gs_d[:])
            us_t = pscale.tile([128, ot_g], f32, name="us_t")
            nc.sync.dma_start(out=us_t[:], in_=us_d[:])

            def mm3(ps, wt, xt, ko):
                """3-pass fp8 DoubleRow product into psum ps.

                wt: [128, 2, ko, 128] (dim1: 0=w_hi2, 1=w_lo8)
                xt: [128, 2, ko, t]   (dim1: 0=a_hi8, 1=a_lo8)
                """
                np_ = ko // 2
                for j in range(np_):
                    k = slice(2 * j, 2 * j + 2)
                    nc.tensor.matmul(
                        ps[:], wt[:, 0, k, :], xt[:, 0, k, :],
                        start=(j == 0), stop=False, perf_mode=DR,
                    )
                for j in range(np_):
                    k = slice(2 * j, 2 * j + 2)
                    nc.tensor.matmul(
                        ps[:], wt[:, 0, k, :], xt[:, 1, k, :],
                        start=False, stop=False, perf_mode=DR,
                    )
                for j in range(np_):
                    k = slice(2 * j, 2 * j + 2)
                    nc.tensor.matmul(
                        ps[:], wt[:, 1, k, :], xt[:, 0, k, :],
                        start=False, stop=(j == np_ - 1), perf_mode=DR,
                    )

            def g_group(m, ot, xt, hh, hl):
                """Gate+up matmul group for (mega m, out tile ot) + SwiGLU."""
                gw = pw.tile([128, 2, ko_g, 128], f8, tag="gw", name="gw")
                nc.sync.dma_start(out=gw[:], in_=gw_d[ot])
                uw = pw.tile([128, 2, ko_g, 128], f8, tag="uw", name="uw")
                nc.sync.dma_start(out=uw[:], in_=uw_d[ot])

                psg = pp.tile([128, t_mega], f32, tag="ps", name="psg")
                mm3(psg, gw, xt, ko_g)
                psu = pp.tile([128, t_mega], f32, tag="ps", name="psu")
                mm3(psu, uw, xt, ko_g)

                gact = pe.tile([128, t_mega], f32, tag="gact", name="gact")
                nc.scalar.activation(
                    gact[:], psg[:], silu, scale=gs_t[:, ot : ot + 1]
                )
                # h = (up_psum * up_scale) * silu(gate * gate_scale), f32
                h32 = pe.tile([128, t_mega], f32, tag="h32", name="h32")
                nc.vector.scalar_tensor_tensor(
                    h32[:], psu[:], us_t[:, ot : ot + 1], gact[:], mult, mult
                )
                # split h into fp8 hi/lo at scale 8
                nc.scalar.activation(hh[:, ot, :], h32[:], copy_act, scale=8.0)
                nc.vector.scalar_tensor_tensor(
                    hl[:, ot, :], h32[:], 8.0, hh[:, ot, :], mult, sub
                )

            def d_group(m, o2, hh, hl):
                """Down matmul group for (mega m, out tile o2); host scales."""
                dw = pdw.tile([128, 2, ko_d, 128], f8, tag="dw", name="dw")
                nc.sync.dma_start(out=dw[:], in_=dw_d[o2])
                pso = pp.tile([128, t_mega], f32, tag="ps", name="pso")
                np_ = ko_d // 2
                for j in range(np_):
                    k = slice(2 * j, 2 * j + 2)
                    nc.tensor.matmul(
                        pso[:], dw[:, 0, k, :], hh[:, k, :],
                        start=(j == 0), stop=False, perf_mode=DR,
                    )
                for j in range(np_):
                    k = slice(2 * j, 2 * j + 2)
                    nc.tensor.matmul(
                        pso[:], dw[:, 0, k, :], hl[:, k, :],
                        start=False, stop=False, perf_mode=DR,
                    )
                for j in range(np_):
                    k = slice(2 * j, 2 * j + 2)
                    nc.tensor.matmul(
                        pso[:], dw[:, 1, k, :], hh[:, k, :],
                        start=False, stop=(j == np_ - 1), perf_mode=DR,
                    )
                ob = po.tile([128, t_mega], f32, tag="ob", name="ob")
                nc.scalar.copy(ob[:], pso[:])
                nc.sync.dma_start(
                    out=out_d[
                        o2 * 128 : (o2 + 1) * 128,
                        m * t_mega : (m + 1) * t_mega,
                    ],
                    in_=ob[:],
                )

            # Software pipeline: interleave mega m's gate/up groups with mega
            # m-1's down groups, spreading the down-phase DMA (down weights +
            # out stores) across the whole mega so HBM never saturates and the
            # PE never stalls.
            prev = None  # (m-1, hh, hl)
            for m in range(n_mega):
                xt = px.tile([128, 2, ko_g, t_mega], f8, tag="xt", name="xt")
                nc.sync.dma_start(out=xt[:], in_=x_d[m])
                hh = ph.tile([128, ko_d, t_mega], f8, tag="hh", name="hh")
                hl = ph.tile([128, ko_d, t_mega], f8, tag="hl", name="hl")

                for ot in range(ot_g):
                    g_group(m, ot, xt, hh, hl)
                    if prev is not None:
                        pm, phh, phl = prev
                        for o2 in range(
                            ot_d * ot // ot_g, ot_d * (ot + 1) // ot_g
                        ):
                            d_group(pm, o2, phh, phl)
                prev = (m, hh, hl)

            pm, phh, phl = prev
            for o2 in range(ot_d):
                d_group(pm, o2, phh, phl)

    nc.compile()
    return nc


def _get_module(t_mega, n_mega, ko_g, ot_g, ot_d):
    key = (t_mega, n_mega, ko_g, ot_g, ot_d)
    if key not in _nc_cache:
        _nc_cache[key] = _build_module(t_mega, n_mega, ko_g, ot_g, ot_d)
    return _nc_cache[key]


def _prep_x(x, t_mega, n_mega, ko_g):
    """[T, H] f32 -> tiled [mega, ki, 2(hi/lo), ko, t] fp8 activations."""
    t_total = n_mega * t_mega
    xf = np.ascontiguousarray(x.reshape(t_total, ko_g * 128), dtype=np.float32)
    xr = xf.reshape(n_mega, t_mega, ko_g, 128).transpose(0, 3, 2, 1)
    x8 = 8.0 * np.ascontiguousarray(xr)
    x_hi = x8.astype(_E4)
    x_lo = (x8 - x_hi.astype(np.float32)).astype(_E4)
    return np.ascontiguousarray(np.stack([x_hi, x_lo], axis=2))


def _prep_w(w_int, ot, ko):
    """[ot*128 (o), ko*128 (k)] int-valued -> [ot, ki, 2, ko, o] fp8.

    dim2: 0 = 2*round(w/16) (exact e4m3), 1 = (w - 16*round(w/16))/8.
    """
    w = w_int.astype(np.float32).reshape(ot, 128, ko, 128).transpose(0, 3, 2, 1)
    w_hi = np.round(w * (1.0 / 16.0))
    w_hi2 = (2.0 * w_hi).astype(_E4)
    w_lo8 = ((w - 16.0 * w_hi) * 0.125).astype(_E4)
    return np.ascontiguousarray(np.stack([w_hi2, w_lo8], axis=2))


def _prep_scale(s, ot):
    return np.ascontiguousarray(s.reshape(ot, 128).T, dtype=np.float32)


def _run_spmd(nc, in_maps, trace):
    from concourse.bass_utils import run_bass_kernel_spmd

    return run_bass_kernel_spmd(
        nc, in_maps, core_ids=list(range(len(in_maps))), trace=trace
    )


def kernel(x, gate_wq, gate_scale, up_wq, up_scale, down_wq, down_scale):
    n_mega = T // T_MEGA
    ko_g = H // 128
    ot_g = I_LOC // 128
    ot_d = H // 128

    nc = _get_module(T_MEGA, n_mega, ko_g, ot_g, ot_d)

    x8 = _prep_x(np.asarray(x), T_MEGA, n_mega, ko_g)
    gate_wq = np.asarray(gate_wq)
    up_wq = np.asarray(up_wq)
    down_wq = np.asarray(down_wq)
    gate_scale = np.asarray(gate_scale, dtype=np.float32)
    up_scale = np.asarray(up_scale, dtype=np.float32)
    down_scale = np.asarray(down_scale, dtype=np.float32)

    in_maps = []
    for c in range(NCORES):
        sl = slice(c * I_LOC, (c + 1) * I_LOC)
        im = {
            "x8": x8,
            "gate_w": _prep_w(gate_wq[sl], ot_g, ko_g),
            "up_w": _prep_w(up_wq[sl], ot_g, ko_g),
            "down_w": _prep_w(down_wq[:, sl], ot_d, ot_g),
            "gate_s": _prep_scale(gate_scale[sl], ot_g),
            "up_s": _prep_scale(up_scale[sl], ot_g),
        }
        in_maps.append(im)

    trace = bool(int(os.environ.get("TRNMLP_TRACE", "0")))
    res = _run_spmd(nc, in_maps, trace)
    if trace:
        kernel.last_results = res

    acc = res.results[0]["out"].astype(np.float32, copy=True)
    for r in res.results[1:]:
        acc += r["out"]
    acc *= down_scale[:, None]
    return np.ascontiguousarray(acc.T).reshape(B, S, H).astype(np.float32)


kernel.last_results = None


# revision 4
# speedup vs baseline: 1.6566x; 1.6566x over previous
"""Trainium2 Bass kernel: Mistral quantized MLP (SwiGLU with int8-valued int32
weights, per-output-channel scales).

  gate = (x @ dequant(gate_wq).T), up = (x @ dequant(up_wq).T)
  h = silu(gate) * up
  out = h @ dequant(down_wq).T

Strategy (8 NeuronCores, tensor-parallel on the intermediate dim I):
  - Core c owns rows [c*I/8, (c+1)*I/8) of gate/up and the matching columns of
    down. Each core computes a full [H, T] partial of the down projection;
    the host sums the 8 partials (the "all-reduce"), applies down_scale, and
    transposes back to [B, S, H].

  - Precision: fp16 activations (11-bit significand) with exactly-represented
    int8 weights; one matmul pass for most of the contraction. The LAST
    N8G/N8U k-tiles (of 32) of the gate/up contractions run as single-pass
    fp8(e4m3) DoubleRow matmuls instead: both operands quantized to e4m3
    (x at scale 8, w at scale 1/8), two 128-k-tiles per instruction at 2x
    PE rate. On TRN2 hardware a DoubleRow matmul of K=256 costs the same
    cycles as an fp16 matmul of K=128, so each fp8 k-tile runs at half
    cost; with N8G=N8U=4 this trims ~4% of total PE cycles for ~1.8e-2
    end-to-end L2 error (budget 2e-2; fp16-only measures 3.6e-4).

  - Device layout keeps features on partitions, tokens on the free dim:
    x is pre-transposed/tiled on the host to [mega, ki, ko, t]; weights to
    [o_tile, ki, ko, o] so each DMA is contiguous and each matmul is
    lhsT=[128 k, 128 o] stationary x rhs=[128 k, 512 t] moving, fp32 PSUM.
    The fp8 tails are separate dram tensors in the same layout.
"""

import os

import ml_dtypes
import numpy as np

_E4 = ml_dtypes.float8_e4m3

# Problem dims (hardcoded per the task contract).
B, S, H, I = 2, 2048, 4096, 14336
NCORES = 8
I_LOC = I // NCORES  # 1792
T = B * S  # 4096
T_MEGA = 512  # tokens per resident x block (and per-matmul free dim)

# Number of trailing gate/up k-tiles (of H/128=32) computed in fp8 DoubleRow.
# Must be even (DoubleRow consumes pairs); N8U <= N8G.
N8G = int(os.environ.get("TRNMLP_N8G", "4"))
N8U = int(os.environ.get("TRNMLP_N8U", "4"))

_nc_cache = {}


def _build_module(t_mega, n_mega, ko_g, ot_g, ot_d, n8g, n8u, act_name="Silu"):
    """Build + compile the (SPMD, identical on all cores) Bass module.

    ko_g: contraction tiles for gate/up (H/128)
    ot_g: output tiles per core for gate/up (I_loc/128); also the down
          contraction tile count
    ot_d: output tiles for down (H/128)
    n8g/n8u: trailing gate/up k-tiles in fp8 (even, n8u <= n8g)
    """
    import concourse.tile as tile
    from concourse import bacc, mybir

    f32 = mybir.dt.float32
    f16 = mybir.dt.float16
    f8 = mybir.dt.float8e4
    silu = getattr(mybir.ActivationFunctionType, act_name)
    mult = mybir.AluOpType.mult
    DR = mybir.MatmulPerfMode.DoubleRow
    ko_d = ot_g
    ko16 = ko_g - n8g  # fp16 k-tiles for gate (and up: ko_g - n8u)
    assert n8g % 2 == 0 and n8u % 2 == 0 and 0 <= n8u <= n8g

    nc = bacc.Bacc(
        "TRN2",
        target_bir_lowering=False,
        debug=False,
        enable_asserts=False,
        num_devices=NCORES,
    )

    xh_d = nc.dram_tensor(
        "x_hi", [n_mega, 128, ko16 + (n8g - n8u), t_mega], f16,
        kind="ExternalInput",
    ).ap()
    x8_d = None
    if n8g:
        x8_d = nc.dram_tensor(
            "x8", [n_mega, 128, n8g, t_mega], f8, kind="ExternalInput"
        ).ap()
    gw_d = nc.dram_tensor(
        "gate_w", [ot_g, 128, ko16, 128], f16, kind="ExternalInput"
    ).ap()
    uw_d = nc.dram_tensor(
        "up_w", [ot_g, 128, ko_g - n8u, 128], f16, kind="ExternalInput"
    ).ap()
    gw8_d = uw8_d = None
    if n8g:
        gw8_d = nc.dram_tensor(
            "gate_w8", [ot_g, 128, n8g, 128], f8, kind="ExternalInput"
        ).ap()
    if n8u:
        uw8_d = nc.dram_tensor(
            "up_w8", [ot_g, 128, n8u, 128], f8, kind="ExternalInput"
        ).ap()
    dw_d = nc.dram_tensor(
        "down_w", [ot_d, 128, ko_d, 128], f16, kind="ExternalInput"
    ).ap()
    gs_d = nc.dram_tensor("gate_s", [128, ot_g], f32, kind="ExternalInput").ap()
    us_d = nc.dram_tensor("up_s", [128, ot_g], f32, kind="ExternalInput").ap()
    out_d = nc.dram_tensor(
        "out", [ot_d * 128, n_mega * t_mega], f32, kind="ExternalOutput"
    ).ap()

    with tile.TileContext(nc) as tc:
        with (
            tc.tile_pool(name="px", bufs=2) as px,
            tc.tile_pool(name="pw", bufs=2) as pw,
            tc.tile_pool(name="pdw", bufs=4) as pdw,
            tc.tile_pool(name="ph", bufs=2) as ph,
            tc.tile_pool(name="pe", bufs=2) as pe,
            tc.tile_pool(name="po", bufs=3) as po,
            tc.tile_pool(name="pscale", bufs=1) as pscale,
            tc.tile_pool(name="pp", bufs=8, space="PSUM") as pp,
        ):
            gs_t = pscale.tile([128, ot_g], f32, name="gs_t")
            nc.sync.dma_start(out=gs_t[:], in_=gs_d[:])
            us_t = pscale.tile([128, ot_g], f32, name="us_t")
            nc.sync.dma_start(out=us_t[:], in_=us_d[:])

            def g_group(m, ot, xh, x8, hh):
                """Gate+up matmul group for (mega m, out tile ot) + SwiGLU."""
                gw = pw.tile([128, ko16, 128], f16, tag="gw", name="gw")
                nc.sync.dma_start(out=gw[:], in_=gw_d[ot])
                uw = pw.tile(
                    [128, ko_g - n8u, 128], f16, tag="uw", name="uw"
                )
                nc.sync.dma_start(out=uw[:], in_=uw_d[ot])
                if n8g:
                    gw8 = pw.tile([128, n8g, 128], f8, tag="gw8", name="gw8")
                    nc.sync.dma_start(out=gw8[:], in_=gw8_d[ot])
                if n8u:
                    uw8 = pw.tile([128, n8u, 128], f8, tag="uw8", name="uw8")
                    nc.sync.dma_start(out=uw8[:], in_=uw8_d[ot])

                psg = pp.tile([128, t_mega], f32, tag="ps", name="psg")
                for k in range(ko16):
                    nc.tensor.matmul(
                        psg[:], gw[:, k, :], xh[:, k, :],
                        start=(k == 0), stop=(n8g == 0 and k == ko16 - 1),
                    )
                for p in range(n8g // 2):
                    sl = slice(2 * p, 2 * p + 2)
                    nc.tensor.matmul(
                        psg[:], gw8[:, sl, :], x8[:, sl, :],
                        start=False, stop=(p == n8g // 2 - 1),
                        perf_mode=DR,
                    )
                psu = pp.tile([128, t_mega], f32, tag="ps", name="psu")
                for k in range(ko_g - n8u):
                    nc.tensor.matmul(
                        psu[:], uw[:, k, :], xh[:, k, :],
                        start=(k == 0), stop=(n8u == 0 and k == ko_g - n8u - 1),
                    )
                for p in range(n8u // 2):
                    # up's fp8 tiles are the LAST n8u of x8
                    sl = slice(n8g - n8u + 2 * p, n8g - n8u + 2 * p + 2)
                    nc.tensor.matmul(
                        psu[:], uw8[:, sl.start - (n8g - n8u) : sl.stop - (n8g - n8u), :],
                        x8[:, sl, :],
                        start=False, stop=(p == n8u // 2 - 1),
                        perf_mode=DR,
                    )

                gact = pe.tile([128, t_mega], f32, tag="gact", name="gact")
                nc.scalar.activation(
                    gact[:], psg[:], silu, scale=gs_t[:, ot : ot + 1]
                )
                # h = (up_psum * up_scale) * silu(gate * gate_scale)
                nc.vector.scalar_tensor_tensor(
                    hh[:, ot, :], psu[:], us_t[:, ot : ot + 1], gact[:],
                    mult, mult,
                )

            def d_group(m, o2, hh):
                """Down matmul group for (mega m, out tile o2); host scales."""
                dw = pdw.tile([128, ko_d, 128], f16, tag="dw", name="dw")
                nc.sync.dma_start(out=dw[:], in_=dw_d[o2])
                pso = pp.tile([128, t_mega], f32, tag="ps", name="pso")
                for k in range(ko_d):
                    nc.tensor.matmul(
                        pso[:], dw[:, k, :], hh[:, k, :],
                        start=(k == 0), stop=(k == ko_d - 1),
                    )
                ob = po.tile([128, t_mega], f32, tag="ob", name="ob")
                nc.scalar.copy(ob[:], pso[:])
                nc.sync.dma_start(
                    out=out_d[
                        o2 * 128 : (o2 + 1) * 128,
                        m * t_mega : (m + 1) * t_mega,
                    ],
                    in_=ob[:],
                )

            # Software pipeline: interleave mega m's gate/up groups with mega
            # m-1's down groups, spreading the down-phase DMA (down weights +
            # out stores) across the whole mega so HBM never saturates and the
            # PE never stalls.
            prev = None  # (m-1, hh)
            for m in range(n_mega):
                xh = px.tile(
                    [128, ko16 + (n8g - n8u), t_mega], f16, tag="xh", name="xh"
                )
                nc.sync.dma_start(out=xh[:], in_=xh_d[m])
                x8 = None
                if n8g:
                    x8 = px.tile([128, n8g, t_mega], f8, tag="x8", name="x8")
                    nc.sync.dma_start(out=x8[:], in_=x8_d[m])
                hh = ph.tile([128, ko_d, t_mega], f16, tag="hh", name="hh")

                for ot in range(ot_g):
                    g_group(m, ot, xh, x8, hh)
                    if prev is not None:
                        pm, phh = prev
                        for o2 in range(
                            ot_d * ot // ot_g, ot_d * (ot + 1) // ot_g
                        ):
                            d_group(pm, o2, phh)
                prev = (m, hh)

            pm, phh = prev
            for o2 in range(ot_d):
                d_group(pm, o2, phh)

    nc.compile()
    return nc


def _get_module(t_mega, n_mega, ko_g, ot_g, ot_d, n8g, n8u):
    key = (t_mega, n_mega, ko_g, ot_g, ot_d, n8g, n8u)
    if key not in _nc_cache:
        _nc_cache[key] = _build_module(
            t_mega, n_mega, ko_g, ot_g, ot_d, n8g, n8u
        )
    return _nc_cache[key]


def _prep_x(x, t_mega, n_mega, ko_g, n8g, n8u):
    """[T, H] f32 -> ([mega, ki, ko16+(n8g-n8u), t] f16, [mega, ki, n8g, t] f8).

    The fp16 tensor covers k-tiles [0, ko_g - n8u); the fp8 tensor covers the
    last n8g tiles (gate uses all of them, up the last n8u; tiles in
    [ko16, ko16 + n8g - n8u) appear in BOTH, consumed as fp16 by up and as
    fp8 by gate).
    """
    ko16 = ko_g - n8g
    t_total = n_mega * t_mega
    xf = np.ascontiguousarray(x.reshape(t_total, ko_g * 128), dtype=np.float32)
    xr = xf.reshape(n_mega, t_mega, ko_g, 128).transpose(0, 3, 2, 1)
    x_hi = np.ascontiguousarray(xr[:, :, : ko_g - n8u, :]).astype(np.float16)
    x8 = None
    if n8g:
        x8 = (8.0 * np.ascontiguousarray(xr[:, :, ko16:, :])).astype(_E4)
    return x_hi, x8


def _prep_w(w_int, ot, ko, n8):
    """[ot*128 (o), ko*128 (k)] int-valued -> fp16 [ot, ki, ko-n8, o] plus
    fp8 [ot, ki, n8, o] (= w/8 on the last n8 k-tiles)."""
    w = w_int.astype(np.float32).reshape(ot, 128, ko, 128).transpose(0, 3, 2, 1)
    w16 = np.ascontiguousarray(w[:, :, : ko - n8, :]).astype(np.float16)
    if n8 == 0:
        return w16, None
    w8 = (np.ascontiguousarray(w[:, :, ko - n8 :, :]) * 0.125).astype(_E4)
    return w16, w8


def _prep_scale(s, ot):
    return np.ascontiguousarray(s.reshape(ot, 128).T, dtype=np.float32)


def _run_spmd(nc, in_maps, trace):
    from concourse.bass_utils import run_bass_kernel_spmd

    return run_bass_kernel_spmd(
        nc, in_maps, core_ids=list(range(len(in_maps))), trace=trace
    )


def kernel(x, gate_wq, gate_scale, up_wq, up_scale, down_wq, down_scale):
    n_mega = T // T_MEGA
    ko_g = H // 128
    ot_g = I_LOC // 128
    ot_d = H // 128

    nc = _get_module(T_MEGA, n_mega, ko_g, ot_g, ot_d, N8G, N8U)

    x_hi, x8 = _prep_x(np.asarray(x), T_MEGA, n_mega, ko_g, N8G, N8U)
    gate_wq = np.asarray(gate_wq)
    up_wq = np.asarray(up_wq)
    down_wq = np.asarray(down_wq)
    gate_scale = np.asarray(gate_scale, dtype=np.float32)
    up_scale = np.asarray(up_scale, dtype=np.float32)
    down_scale = np.asarray(down_scale, dtype=np.float32)

    in_maps = []
    for c in range(NCORES):
        sl = slice(c * I_LOC, (c + 1) * I_LOC)
        gw16, gw8 = _prep_w(gate_wq[sl], ot_g, ko_g, N8G)
        uw16, uw8 = _prep_w(up_wq[sl], ot_g, ko_g, N8U)
        dw16, _ = _prep_w(down_wq[:, sl], ot_d, ot_g, 0)
        im = {
            "x_hi": x_hi,
            "gate_w": gw16,
            "up_w": uw16,
            "down_w": dw16,
            "gate_s": _prep_scale(gate_scale[sl], ot_g),
            "up_s": _prep_scale(up_scale[sl], ot_g),
        }
        if x8 is not None:
            im["x8"] = x8
        if gw8 is not None:
            im["gate_w8"] = gw8
        if uw8 is not None:
            im["up_w8"] = uw8
        in_maps.append(im)

    trace = bool(int(os.environ.get("TRNMLP_TRACE", "0")))
    res = _run_spmd(nc, in_maps, trace)
    if trace:
        kernel.last_results = res

    acc = res.results[0]["out"].astype(np.float32, copy=True)
    for r in res.results[1:]:
        acc += r["out"]
    acc *= down_scale[:, None]
    return np.ascontiguousarray(acc.T).reshape(B, S, H).astype(np.float32)


kernel.last_results = None
